# revision 1
# baseline (speedup 1.0000x reference)
"""GSN message-passing kernel for 8 Trainium2 NeuronCores (Bass/Tile).

Math per layer (algebraically reduced from the reference):
    h   = MLP(x)                       # 3->16->16->3, LeakyReLU(0.01)
    g   = dinv * h                     # dinv = rsqrt(in-degree), 0 if deg==0
    s   = segment_sum(g[row], col)     # the only sparse op
    out = x + dinv*s - (dinv*t)*h      # t[w] = sum_{e:col=w} dinv[row_e]

deg, dinv, t depend only on edge_index and are host-precomputed.
Sharding: targets (col) range-sharded across the 8 cores. Each core's shard
targets are relabeled by in-degree desc so the k-th-in-edge gather "planes"
are contiguous prefixes (no wasted gather slots). Per layer each core MLPs
its own shard, the g-tables are AllGathered, then each core gathers its
planes (indirect DMA, 128 rows/call), accumulates, and combines. The host
concatenates shard outputs and un-permutes the relabeling.
"""
import sys
sys.path.insert(0, "/opt/trn_rl_repo")

import numpy as np

import concourse.bass as bass
import concourse.tile as tile
from concourse import mybir
from concourse.vector_clock import ScopedClock

N_VERTS = 1_000_000
N_CORES = 8
P = 128
NEG = 0.01

SHARD = N_VERTS // N_CORES            # 125000 targets per core
COLS = (SHARD + P - 1) // P           # 977 free columns
SHARD_PAD = COLS * P                  # 125056 incl. dummy targets
TABLE_ROWS = N_CORES * SHARD_PAD      # 1000448
TABLE_PAD = TABLE_ROWS + P            # + zero tail for plane padding

MAX_WAITS = 1
_wfix = [0]


def _drain_and_barrier_fixed(self, tick_clock, wait_clock):
    nc = self.nc
    carrier = nc.sync.nop(nofuse=True)
    wait_clock.add_sem_waits(carrier.ins, ScopedClock({None: tick_clock.global_clock}))
    si = carrier.ins.sync_info
    ow = list(si.on_wait) if si is not None and si.on_wait else []
    if len(ow) > MAX_WAITS:
        carrier.ins.sync_info = mybir.SyncInfo(
            on_wait=ow[:MAX_WAITS], on_update=list(si.on_update or []))
        for i in range(MAX_WAITS, len(ow), MAX_WAITS):
            extra = nc.sync.nop(nofuse=True)
            extra.ins.sync_info = mybir.SyncInfo(
                on_wait=ow[i:i + MAX_WAITS], on_update=[])
    nc.sync.drain()
    nc.all_engine_barrier()
    assert self.sems is not None
    popped = nc._tile_sem_poison_stack.pop()
    assert popped is self._sem_poison
    nc.clear_and_free_semaphores(list(self.sems.allocated().values()))
    nc.all_engine_barrier()


tile.TileContext._drain_and_barrier = _drain_and_barrier_fixed


def fix_waits(nc):
    """This container's walrus lowers at most 1 sync-wait per instruction;
    split excess waits onto same-engine NoOps placed just before."""
    for f in nc.m.functions:
        for b in f.blocks:
            out, changed = [], False
            for inst in b.instructions:
                si = getattr(inst, "sync_info", None)
                ow = list(si.on_wait) if si is not None and si.on_wait else []
                if len(ow) > MAX_WAITS:
                    changed = True
                    excess, keep = ow[:-MAX_WAITS], ow[-MAX_WAITS:]
                    for i in range(0, len(excess), MAX_WAITS):
                        _wfix[0] += 1
                        out.append(mybir.InstNoOp(
                            name=f"WFIX-{_wfix[0]}", engine=inst.engine,
                            ins=[], outs=[],
                            sync_info=mybir.SyncInfo(
                                on_wait=excess[i:i + MAX_WAITS], on_update=[])))
                    inst.sync_info = mybir.SyncInfo(
                        on_wait=keep, on_update=list(si.on_update or []))
                out.append(inst)
            if changed:
                b.instructions = out


def spread_gather_queues(nc, nq=4):
    """Round-robin dynamic-gather DMAs over the qPoolDynamic{i} queues."""
    n = 0
    for f in nc.m.functions:
        for b in f.blocks:
            for inst in b.instructions:
                if (type(inst).__name__ == "InstDMACopy"
                        and getattr(inst, "queue", None) == "qPoolDynamic"
                        and any(getattr(a, "dynamic_ap_info", None) is not None
                                for a in (inst.ins or []))):
                    q = n % nq
                    if q:
                        inst.queue = f"qPoolDynamic{q}"
                    n += 1


# ---------------------------------------------------------------- host prep

def _host_prep(edge_index):
    row = np.asarray(edge_index[0], dtype=np.int64)
    col = np.asarray(edge_index[1], dtype=np.int64)

    deg = np.bincount(col, minlength=N_VERTS).astype(np.float64)
    dinv = np.where(deg > 0, 1.0 / np.sqrt(np.maximum(deg, 1e-12)), 0.0)
    dinv = dinv.astype(np.float32)
    t = np.bincount(col, weights=dinv[row].astype(np.float64),
                    minlength=N_VERTS).astype(np.float32)

    shard_of = col // SHARD
    table_pos = np.empty(N_VERTS, dtype=np.int64)
    per_core = []
    for c in range(N_CORES):
        lo = c * SHARD
        vdeg = deg[lo:lo + SHARD]
        perm = np.argsort(-vdeg, kind="stable")   # w' slot -> original local
        verts_global = perm + lo
        table_pos[verts_global] = c * SHARD_PAD + np.arange(SHARD)
        inv = np.empty(SHARD, dtype=np.int64)
        inv[perm] = np.arange(SHARD)
        per_core.append((lo, verts_global, inv, vdeg[perm]))

    cores = []
    for c in range(N_CORES):
        lo, verts_global, inv, deg_sorted = per_core[c]
        m = shard_of == c
        e_row, e_col = row[m], col[m]
        wprime = inv[e_col - lo]
        o2 = np.lexsort((e_row, wprime))
        wp_s, row_s = wprime[o2], e_row[o2]
        first = np.r_[True, wp_s[1:] != wp_s[:-1]] if len(wp_s) else np.array([], bool)
        idx_all = np.arange(len(wp_s))
        start = np.maximum.accumulate(np.where(first, idx_all, 0))
        rank = idx_all - start
        kmax = int(deg_sorted[0]) if len(deg_sorted) else 0
        planes = []
        for k in range(kmax):
            mk = int(np.searchsorted(-deg_sorted, -(k + 1), side="right"))
            ncalls = (mk + P - 1) // P
            src = np.full(ncalls * P, TABLE_ROWS, dtype=np.int32)  # zero row
            sel = rank == k
            src[wp_s[sel]] = table_pos[row_s[sel]].astype(np.int32)
            planes.append(src.reshape(ncalls, P).T.copy())  # [P, ncalls]
        cores.append({
            "verts_global": verts_global,
            "planes": planes,
            "dinv": dinv[verts_global],
            "c1": (dinv * t)[verts_global],
        })
    return cores


def _pad_layout(vals):
    """[SHARD(,D)] in w'-order -> [P, COLS*D], slot w'=i*P+p at [p, i*D+d]."""
    D = 1 if vals.ndim == 1 else vals.shape[1]
    buf = np.zeros((SHARD_PAD, D), dtype=np.float32)
    buf[:SHARD] = np.asarray(vals, np.float32).reshape(SHARD, D)
    return np.ascontiguousarray(
        buf.reshape(COLS, P, D).transpose(1, 0, 2).reshape(P, COLS * D))


# ------------------------------------------------------------ device kernel

def _build_kernel(ncalls_per_plane, weights):
    nc = bass.Bass(num_swdge_queues=4)
    dt = mybir.dt.float32
    Add, Mul, Max, Sub = (mybir.AluOpType.add, mybir.AluOpType.mult,
                          mybir.AluOpType.max, mybir.AluOpType.subtract)

    x_in = nc.declare_dram_parameter("x0", [P, COLS * 3], dt, isOutput=False)
    dinv_in = nc.declare_dram_parameter("dinv", [P, COLS], dt, isOutput=False)
    c1_in = nc.declare_dram_parameter("c1", [P, COLS], dt, isOutput=False)
    out_ext = nc.declare_dram_parameter("out", [P, COLS * 3], dt, isOutput=True)
    plane_in = [nc.declare_dram_parameter(f"plane{k}", [P, n], mybir.dt.int32,
                                          isOutput=False)
                for k, n in enumerate(ncalls_per_plane)]

    W = {k: np.asarray(w, np.float64) for k, w in weights.items()}
    CH = 489  # MLP column chunk (SBUF budget)

    with tile.TileContext(nc) as tc:
        with (tc.tile_pool(name="sb", bufs=1) as sb,
              tc.tile_pool(name="gp", bufs=4) as gp,
              tc.tile_pool(name="gg", bufs=64) as gg,
              tc.tile_pool(name="dram", bufs=1, space="DRAM") as dram):
            x = sb.tile([P, COLS, 3], dt)
            dinv = sb.tile([P, COLS], dt)
            c1 = sb.tile([P, COLS], dt)
            h = sb.tile([P, COLS, 3], dt)
            hid = sb.tile([P, CH, 16], dt)
            hid2 = sb.tile([P, CH, 16], dt)
            s = sb.tile([P, COLS, 3], dt)
            gsh = sb.tile([P, COLS, 3], dt)
            ztail = sb.tile([P, 3], dt)
            gsh_b = dram.tile([SHARD_PAD, 3], dt)
            table_b = dram.tile([TABLE_PAD, 3], dt)

            G = 512  # slots (free columns) gathered per indirect call

            nc.sync.dma_start(out=x[:], in_=x_in[:, :])
            nc.sync.dma_start(out=dinv[:], in_=dinv_in[:, :])
            nc.sync.dma_start(out=c1[:], in_=c1_in[:, :])
            nc.vector.memset(ztail[:], 0.0)
            nc.sync.dma_start(out=table_b[TABLE_ROWS:TABLE_PAD, :], in_=ztail[:])

            def dense(src, dst, Wm, cw, n_in, n_out):
                for j in range(n_out):
                    nc.vector.tensor_scalar_mul(
                        dst[:, :cw, j], src[:, :cw, 0], float(Wm[0][j]))
                    for ci in range(1, n_in):
                        nc.vector.scalar_tensor_tensor(
                            out=dst[:, :cw, j], in0=src[:, :cw, ci],
                            scalar=float(Wm[ci][j]), in1=dst[:, :cw, j],
                            op0=Mul, op1=Add)

            def leaky(t_, cw, n):
                nc.vector.scalar_tensor_tensor(
                    out=t_[:, :cw, :n], in0=t_[:, :cw, :n], scalar=NEG,
                    in1=t_[:, :cw, :n], op0=Mul, op1=Max)

            def mlp(lname):
                W1, W2, W3 = W[lname + "1"], W[lname + "2"], W[lname + "3"]
                for c0 in range(0, COLS, CH):
                    cw = min(CH, COLS - c0)
                    xs = x[:, c0:c0 + cw, :]
                    dense(xs, hid, W1, cw, 3, 16)
                    leaky(hid, cw, 16)
                    dense(hid, hid2, W2, cw, 16, 16)
                    leaky(hid2, cw, 16)
                    dense(hid2, h[:, c0:c0 + cw, :], W3, cw, 16, 3)

            for layer in ("a", "b"):
                mlp(layer)
                for j in range(3):
                    nc.vector.tensor_mul(gsh[:, :, j], h[:, :, j], dinv[:])
                nc.sync.dma_start(
                    out=gsh_b[:].rearrange("(i p) d -> p i d", p=P),
                    in_=gsh[:])
                nc.gpsimd.collective_compute(
                    "AllGather", mybir.AluOpType.bypass,
                    replica_groups=[list(range(N_CORES))],
                    ins=[gsh_b[:].opt()],
                    outs=[table_b[0:TABLE_ROWS, :].opt()],
                )
                nc.vector.memset(s[:], 0.0)
                for k, n in enumerate(ncalls_per_plane):
                    for i0 in range(0, n, G):
                        g_ = min(G, n - i0)
                        tix = gp.tile([P, g_], mybir.dt.int32, tag="ix")
                        nc.sync.dma_start(out=tix[:],
                                          in_=plane_in[k][:, i0:i0 + g_])
                        for i in range(g_):
                            nc.gpsimd.indirect_dma_start(
                                out=s[:, i0 + i, :], out_offset=None,
                                in_=table_b[:],
                                in_offset=bass.IndirectOffsetOnAxis(
                                    ap=tix[:, i:i + 1], axis=0),
                                compute_op=mybir.AluOpType.add)
                for j in range(3):
                    nc.vector.tensor_mul(s[:, :, j], s[:, :, j], dinv[:])
                    nc.vector.tensor_mul(h[:, :, j], h[:, :, j], c1[:])
                nc.vector.tensor_add(x[:], x[:], s[:])
                nc.vector.tensor_tensor(out=x[:], in0=x[:], in1=h[:], op=Sub)

            nc.sync.dma_start(out=out_ext[:, :], in_=x[:])
    return nc


# ----------------------------------------------------------------- entry

def kernel(verts, edge_index, W1_0, W2_0, W3_0, W1_1, W2_1, W3_1):
    verts = np.asarray(verts, dtype=np.float32)
    cores = _host_prep(np.asarray(edge_index))

    kmax = max(len(c["planes"]) for c in cores)
    ncalls = [max((c["planes"][k].shape[1] if k < len(c["planes"]) else 0)
                  for c in cores) for k in range(kmax)]
    weights = {"a1": np.asarray(W1_0), "a2": np.asarray(W2_0),
               "a3": np.asarray(W3_0), "b1": np.asarray(W1_1),
               "b2": np.asarray(W2_1), "b3": np.asarray(W3_1)}
    nc = _build_kernel(ncalls, weights)
    spread_gather_queues(nc)
    fix_waits(nc)

    in_maps = []
    for c in cores:
        m = {"x0": _pad_layout(verts[c["verts_global"]]),
             "dinv": _pad_layout(c["dinv"]),
             "c1": _pad_layout(c["c1"])}
        for k in range(kmax):
            pl = (c["planes"][k] if k < len(c["planes"])
                  else np.zeros((P, 0), np.int32))
            if pl.shape[1] < ncalls[k]:
                pad = np.full((P, ncalls[k] - pl.shape[1]), TABLE_ROWS, np.int32)
                pl = np.concatenate([pl, pad], axis=1)
            m[f"plane{k}"] = np.ascontiguousarray(pl)
        in_maps.append(m)

    from concourse.bass_utils import run_bass_kernel_spmd
    res = run_bass_kernel_spmd(nc, in_maps, core_ids=list(range(N_CORES)))

    out = np.empty((N_VERTS, 3), dtype=np.float32)
    for ci, c in enumerate(cores):
        o = res.results[ci]["out"].reshape(P, COLS, 3).transpose(1, 0, 2)
        out[c["verts_global"]] = o.reshape(SHARD_PAD, 3)[:SHARD]
    return out

